# revision 43
# baseline (speedup 1.0000x reference)
"""Causal self-attention (B=2, T=2048, C=1024, H=16) on 8 TRN2 NeuronCores.

Sharding: core c -> batch b = c // 4, head group = heads [4*(c%4) .. 4*(c%4)+4).
Each core computes qkv for its 4 heads on its batch, causal attention, and a
row-parallel partial of the output projection (over its 256 head channels).
The host sums the 4 partials per batch (bf16 partials, upcast on host);
b_proj/4 is added on-device so the sum reproduces a single b_proj add.

All device tensors are pre-transposed on the host so the kernel never
transposes on-chip:
  xt   [C, T]    = x[b].T                     (bf16)
  wqkt [C, 512]  = w_attn[qk rows].T          (bf16)  cols: q_h0..q_h3 k_h0..k_h3
  wvt  [C, 256]  = w_attn[v rows].T           (bf16)
  wpt  [256, C]  = w_proj[:, head cols].T     (bf16)
  out_t[C, T]    = partial (x @ w_proj.T).T   (bf16)

Round 3 key insight (HW-measured): row/col-tiled matmul pairs run 2x
CONCURRENT on the PE, but only when no full-128x128 matmul sits between them
(mode switch drains the array).  So the two 64-contraction score matmuls of a
head pair are emitted adjacently, and score pairs for two consecutive kt
tiles are batched per "superstep" before the (128-mode) AV/filler matmuls.
The softmax denominators ride along as a 65th V column; 1/den is computed by
the custom DVE op reciprocal_approx_fast (no ScalarE Ln/Exp chain, no table
pressure) and spread across partitions by a K=1 fp32r matmul.
"""

import os
import sys
import types

import numpy as np
import ml_dtypes

import concourse.bass as bass
import concourse.mybir as mybir
import concourse.tile as tile
from concourse import bacc
from concourse.hw_specs import get_activation_tables

BF16 = ml_dtypes.bfloat16


class _Bacc(bacc.Bacc):
    """Bacc that steers Exp/Ln activations to the combined
    natural_log_exp_and_others table set so the kernel never swaps
    activation tables (set ids keep their act_info.json positions)."""

    def insert_act_table_loads(self):
        import bass_rust as _br
        import concourse.mybir as _mybir

        has_activation = any(
            isinstance(i, _mybir.InstActivation)
            for b in self.main_func.blocks
            for i in b.instructions
        )
        if not has_activation:
            return
        combined = {"natural_log_exp_and_others"}
        steer = {_mybir.ActivationFunctionType.Exp, _mybir.ActivationFunctionType.Ln}
        tables = []
        for name, fns in get_activation_tables(self.m.arch).items():
            if name not in combined:
                fns = {f for f in fns if f not in steer}
            tables.append((name, set(fns)))
        _br.insert_act_table_loads(self, tables)

B, T, C = 2, 2048, 1024
H = 16
DH = 64
N_CORES = 8
HEADS_PER_CORE = 4
TQ = 512          # tq tile (moving dim of scores/AV matmuls)
TK = 128          # tk tile (PSUM partition dim of S^T)
NG = T // TQ      # 4 tq tiles
NKT = T // TK     # 16 tk tiles
NC_ = C // 128    # 8 contraction tiles for the qkv matmuls
FP32 = mybir.dt.float32
F32R = mybir.dt.float32r
BF16_DT = mybir.dt.bfloat16
import os as _os
DEPTH = int(_os.environ.get("K_DEPTH", "8"))
POPS_EVERY = int(_os.environ.get("K_POPS_EVERY", "1"))  # pops per superstep
BOUNDARY_POPS = int(_os.environ.get("K_BPOPS", "3"))
WARMUP_MMS = int(_os.environ.get("K_WARMUP", "8"))
PT_BUFS = int(_os.environ.get("K_PT_BUFS", "10"))
PRO_POPS = int(_os.environ.get("K_PRO_POPS", "1"))      # pops per prologue superstep
RATION = int(_os.environ.get("K_RATION", "0"))          # per-unit pop rationing
EFIRST = int(_os.environ.get("K_EFIRST", "0"))          # normalize e half first
DEFER = int(_os.environ.get("K_DEFER", "0"))            # proj groups deferred 1 unit
MERGEPS = int(_os.environ.get("K_MERGEPS", "0"))        # qv+bp as one 2-buf pool
QTAG = "pj" if MERGEPS else "qv"
BTAG = "pj" if MERGEPS else "bp"
GPRIO = int(_os.environ.get("K_GPRIO", "4"))            # prio chunks on SWDGE


def _ensure_axon_hooks_stub():
    """bass_utils imports antenv.axon_hooks when trace is requested (even via
    the BASS_TRACE env var). The container's antenv stub lacks that module, so
    install a minimal one to keep the no-trace fallback path working."""
    try:
        import antenv  # noqa: F401
    except ImportError:
        return
    if "antenv.axon_hooks" in sys.modules:
        return
    try:
        import antenv.axon_hooks  # noqa: F401
        return
    except ImportError:
        pass
    mod = types.ModuleType("antenv.axon_hooks")
    mod._hook = None

    def set_axon_ntff_profile_hook(h):
        mod._hook = h

    def get_axon_ntff_profile_hook():
        return mod._hook

    mod.set_axon_ntff_profile_hook = set_axon_ntff_profile_hook
    mod.get_axon_ntff_profile_hook = get_axon_ntff_profile_hook
    sys.modules["antenv.axon_hooks"] = mod
    import antenv as _a

    _a.axon_hooks = mod


def build_bass():
    """Emit the single-core SPMD Bass module (same program on all 8 cores)."""
    from collections import deque
    from contextlib import ExitStack

    nc = _Bacc("TRN2", target_bir_lowering=False, debug=False)

    # Blocked DRAM layouts: every [128, X] tile the kernel DMAs is a fully
    # contiguous run in DRAM (1KB+ lines, sequential bursts), instead of the
    # strided / interleaved reads of the natural [C, T] layouts.
    #   xt   [8*4*128, 512]: tile (chunk i, tq block b) at rows (4i+b)*128
    #   wqkt [8*2*128, 256]: tile (chunk i, half h) at rows (2i+h)*128;
    #        half h cols = [q heads (2h,2h+1) | k heads (2h,2h+1)]
    #   out_t[4*8*128, 512]: tile (g, jt) at rows (8g+jt)*128
    xt = nc.declare_dram_parameter("xt", [NC_ * 4 * 128, 512], BF16_DT,
                                   isOutput=False).ap()
    wqkt = nc.declare_dram_parameter("wqkt", [NC_ * 2 * 128, 256], BF16_DT,
                                     isOutput=False).ap()
    wvt = nc.declare_dram_parameter("wvt", [C, 256], BF16_DT, isOutput=False).ap()
    wpt = nc.declare_dram_parameter("wpt", [256, C], BF16_DT, isOutput=False).ap()
    bqk = nc.declare_dram_parameter("bqk", [512, 1], FP32, isOutput=False).ap()
    bv = nc.declare_dram_parameter("bv", [128, 256], FP32, isOutput=False).ap()
    bp = nc.declare_dram_parameter("bp", [C, 1], FP32, isOutput=False).ap()
    out_t = nc.declare_dram_parameter("out_t", [4 * 8 * 128, 512], BF16_DT,
                                      isOutput=True).ap()

    Exp = mybir.ActivationFunctionType.Exp
    mult = mybir.AluOpType.mult
    add = mybir.AluOpType.add
    is_ge = mybir.AluOpType.is_ge

    with tile.TileContext(nc) as tc, ExitStack() as ctx:
        res = ctx.enter_context(tc.tile_pool(name="resident", bufs=1))

        # --- resident loads -------------------------------------------------
        xt_t = [res.tile([128, T], BF16_DT, tag=f"xt{i}", name=f"xt{i}")
                for i in range(NC_)]
        wqk_t = [res.tile([128, 512], BF16_DT, tag=f"wqk{i}", name=f"wqk{i}")
                 for i in range(NC_)]
        wv_t = [res.tile([128, 256], BF16_DT, tag=f"wv{i}", name=f"wv{i}")
                for i in range(NC_)]
        bqk_bt = res.tile([128, 4], FP32, tag="bqkb", name="bqk_bt")
        bqk_t = [bqk_bt[:, j : j + 1] for j in range(4)]
        bp_bt = res.tile([128, 8], FP32, tag="bpb", name="bp_bt")
        bp_t = [bp_bt[:, j : j + 1] for j in range(8)]
        wp_t = [res.tile([128, C], BF16_DT, tag=f"wp{i}", name=f"wp{i}")
                for i in range(2)]
        bv_t = res.tile([128, 256], FP32, tag="bv", name="bv")

        sc_ps = ctx.enter_context(tc.tile_pool(name="sc_ps", bufs=2, space="PSUM"))
        av_ps = ctx.enter_context(tc.tile_pool(name="av_ps", bufs=2, space="PSUM"))
        if MERGEPS:
            # one 2-buf pool for qkv fillers, proj, and the norm broadcast
            # matmuls: norm_post's bc_e no longer serializes behind bc_o's
            # reciprocal (the old bufs=1 bp_ps), and any two of
            # filler/bc/proj can be in flight.  Same 8-bank total.
            qv_ps = bp_ps = ctx.enter_context(
                tc.tile_pool(name="pj_ps", bufs=2, space="PSUM"))
        else:
            qv_ps = ctx.enter_context(
                tc.tile_pool(name="qv_ps", bufs=1, space="PSUM"))
            bp_ps = ctx.enter_context(
                tc.tile_pool(name="bp_ps", bufs=1, space="PSUM"))
        pt_pool = ctx.enter_context(tc.tile_pool(name="pt_pool", bufs=PT_BUFS))
        riv_pool = ctx.enter_context(tc.tile_pool(name="riv", bufs=2))
        bcs_pool = ctx.enter_context(tc.tile_pool(name="bcs", bufs=2))
        scr_pool = ctx.enter_context(tc.tile_pool(name="scr", bufs=2))
        osb_pool = ctx.enter_context(tc.tile_pool(name="osb", bufs=4))

        # --- PE warm-up FIRST: the memset is DVE's first op out of the
        # preamble and the zero matmuls are the PE's first, so the HAM
        # activity ramp (grant ~3.4us after first activity, half-duty clamp
        # after that) anchors as early as possible; everything downstream
        # shifts with it.
        warm_sb = res.tile([128, 512], BF16_DT, tag="warm", name="warm_sb")
        nc.vector.memset(warm_sb[:], 0.0)
        warm_ps = qv_ps.tile([128, 512], FP32, tag=QTAG, name="warm_ps")
        for i in range(WARMUP_MMS):
            nc.tensor.matmul(
                warm_ps[:], lhsT=warm_sb[:, 0:128], rhs=warm_sb[:],
                start=(i == 0), stop=(i == WARMUP_MMS - 1), skip_group_check=True,
            )

        # Priority-ordered input loads, round-robined over both HWDGE queues:
        # everything unit (0,0) and the first qkv groups touch comes first
        # (xt g0-slices, wqk q01/k01 halves, bqk, then wv+bv for the V/AV
        # path), and only then the q23/k23 halves and bp.
        # SBUF wqk tile cols are [q01 | k01 | q23 | k23] so each half is one
        # contiguous [128, 256] transfer from the blocked DRAM layout.
        def _wqk_half(i, off):
            sb = wqk_t[i][:, 256 * off : 256 * (off + 1)]
            dr = wqkt[(2 * i + off) * 128 : (2 * i + off + 1) * 128, :]
            return sb, dr

        # bqk is 2KB and gates the first qk group's bias-add (which in turn
        # frees the PSUM tile for the next group) -> load it first.
        prio = [(bqk_bt[:], bqk.rearrange("(j p) o -> p (j o)", p=128))]
        # xt/wqk pairs interleaved in an [x,w,w,x] pattern: with the n%2
        # queue round-robin below, each HWDGE queue alternates xt and wqk
        # transfers, so contraction-chunk i's (xt, wqk) pair is complete
        # after ~i serial slots instead of the wqk half waiting behind all
        # eight xt transfers.  The last GPRIO chunks ride the SWDGE queue
        # instead (~200GB/s vs ~70GB/s per HWDGE queue), so the full
        # priority set (all 8 chunks needed by the first qk group) lands
        # ~2us sooner.
        gp_prio = []
        for i in range(NC_):
            pair = [(xt_t[i][:, 0:512], xt[4 * i * 128 : (4 * i + 1) * 128, :]),
                    _wqk_half(i, 0)]
            if i >= NC_ - GPRIO:
                gp_prio.extend(pair)
            else:
                prio.extend(pair if i % 2 == 0 else pair[::-1])
        for i in range(NC_):
            prio.append(_wqk_half(i, 1))
        prio.append((bp_bt[:], bp.rearrange("(j p) o -> p (j o)", p=128)))
        # Only Sync and Scalar have HWDGE queues on this part (Vector/Tensor
        # cannot issue DMAs), so the priority stream round-robins over the
        # two of them; gpsimd's SWDGE carries the bulk below.
        for n, (dst, src_ap) in enumerate(prio):
            (nc.sync if n % 2 == 0 else nc.scalar).dma_start(dst, src_ap)
        # SWDGE order: offloaded priority pairs first (qk groups need all 8
        # chunks), then wv/bv for the V path (needed a few us later), then
        # the xt bulk below.
        for dst, src_ap in gp_prio:
            nc.gpsimd.dma_start(dst, src_ap)
        for i in range(NC_):
            nc.gpsimd.dma_start(wv_t[i][:], wvt[128 * i : 128 * (i + 1), :])
        nc.gpsimd.dma_start(bv_t[:], bv[:])

        # Single causal strip mask [128, 128]: keep iff local tq >= local tk.
        maskd = res.tile([128, 128], BF16_DT, tag="maskd", name="maskd")
        nc.gpsimd.memset(maskd[:], 1.0)
        nc.gpsimd.affine_select(
            out=maskd[:], in_=maskd[:], compare_op=is_ge, fill=0.0,
            base=0, pattern=[[1, 128]], channel_multiplier=-1,
        )

        # Stationary for the den-broadcast matmuls, shaped as a full 128x128
        # tile so they run in the default PE mode (no tiling-mode switch):
        # row 64 (the av_* denominator lane) carries ones into out partitions
        # 0-63; everything else is zero.
        zo = res.tile([128, 128], BF16_DT, tag="zo", name="zo")
        nc.gpsimd.memset(zo[:], 0.0)
        nc.vector.memset(zo[64:65, 0:64], 1.0)

        # qT/kT in [head-channel, t] layout: tile p holds heads (2p, 2p+1).
        qk_sb = [
            res.tile([128, T], BF16_DT, tag=f"qk{i}", name=f"qk{i}") for i in range(4)
        ]
        # V natural [t, d] with a ones column after each head: 4*(64+1) cols.
        v_sb = []
        for i in range(NKT):
            t = res.tile([128, 260], BF16_DT, tag=f"v{i}", name=f"v{i}")
            nc.gpsimd.memset(
                t[:].rearrange("p (h c) -> p h c", c=65)[:, :, 64:65], 1.0
            )
            v_sb.append(t)
        att_sb = [
            res.tile([128, T], BF16_DT, tag=f"att{i}", name=f"att{i}")
            for i in range(2)
        ]

        # bulk of xt via the SWDGE path: block 1 first as its own wave (its
        # consumers unblock earliest), then blocks 2+3 merged per chunk (the
        # SWDGE is issue-cadence limited, ~650ns per dma_start).
        for i in range(NC_):
            nc.gpsimd.dma_start(
                xt_t[i][:, 512:1024],
                xt[(4 * i + 1) * 128 : (4 * i + 2) * 128, :],
            )
        for i in range(2):
            nc.gpsimd.dma_start(wp_t[i][:], wpt[128 * i : 128 * (i + 1), :])
        for i in range(NC_):
            nc.gpsimd.dma_start(
                xt_t[i][:, 1024:2048],
                xt[(4 * i + 2) * 128 : (4 * i + 4) * 128, :].rearrange(
                    "(b p) w -> p b w", p=128
                ),
            )

        # pre-fill both rb buffers: later tiles only write lane 64, and the
        # other lanes must hold finite bf16 data (NaN x 0 = NaN in the
        # zero-padded broadcast matmul)
        for i in range(2):
            t0 = riv_pool.tile([128, 512], BF16_DT, tag="rb", name=f"rbinit{i}")
            nc.vector.memset(t0[:], 1.0)

        # --- filler work: qkv projections + output projection --------------
        emitted = set()
        out_dma_n = [0]

        # jt in {0:q01, 1:q23, 2:k01, 3:k23} -> SBUF col block of the
        # half-contiguous [q01 | k01 | q23 | k23] wqk tile layout
        WQK_BLK = [0, 2, 1, 3]

        def emit_qk_group(jt, g, pool=None, tag=QTAG):
            ps = (pool or qv_ps).tile([128, 512], FP32, tag=tag,
                                      name=f"qkps{jt}_{g}")
            wb = WQK_BLK[jt]
            for ct in range(NC_):
                nc.tensor.matmul(
                    ps[:],
                    lhsT=wqk_t[ct][:, 128 * wb : 128 * (wb + 1)],
                    rhs=xt_t[ct][:, TQ * g : TQ * (g + 1)],
                    start=(ct == 0),
                    stop=(ct == NC_ - 1),
                )

            nc.vector.tensor_scalar(
                qk_sb[jt][:, TQ * g : TQ * (g + 1)], ps[:], bqk_t[jt][:], None,
                op0=add,
            )

        def emit_v_group(tt, pool=None, tag=QTAG):
            ps = (pool or qv_ps).tile([128, 512], FP32, tag=tag, name=f"vps{tt}")
            for ct in range(NC_):
                nc.tensor.matmul(
                    ps[:, 0:256],
                    lhsT=xt_t[ct][:, 128 * tt : 128 * (tt + 1)],
                    rhs=wv_t[ct][:],
                    start=(ct == 0),
                    stop=(ct == NC_ - 1),
                )

            vt = v_sb[tt]
            nc.vector.tensor_tensor(
                out=vt[:].rearrange("p (h c) -> p h c", c=65)[:, :, 0:64],
                in0=ps[:, 0:256].rearrange("p (h c) -> p h c", c=64),
                in1=bv_t[:].rearrange("p (h c) -> p h c", c=64),
                op=add,
            )

        Identity = mybir.ActivationFunctionType.Identity

        def emit_proj_group(jt, g, pool=None, tag=BTAG, bias_eng=None):
            pp = (pool or bp_ps).tile([128, 512], FP32, tag=tag, name=f"pj{g}{jt}")
            nc.tensor.matmul(
                pp[:], lhsT=wp_t[0][:, 128 * jt : 128 * (jt + 1)],
                rhs=att_sb[0][:, TQ * g : TQ * (g + 1)], start=True, stop=False,
            )
            nc.tensor.matmul(
                pp[:], lhsT=wp_t[1][:, 128 * jt : 128 * (jt + 1)],
                rhs=att_sb[1][:, TQ * g : TQ * (g + 1)], start=False, stop=True,
            )
            osb = osb_pool.tile([128, 512], BF16_DT, tag="osb", name=f"osb{g}{jt}")
            if bias_eng is nc.scalar:
                # ScalarE bias-add (Identity is in the resident act table) so
                # the epilogue's 8 back-to-back groups don't serialize on DVE
                nc.scalar.activation(osb[:], pp[:], Identity, bias=bp_t[jt][:])
            else:
                nc.vector.tensor_scalar(osb[:], pp[:], bp_t[jt][:], None, op0=add)
            # spread output transfers over DGE queues so the final groups'
            # DMAs drain in parallel instead of serializing; ScalarE's queue
            # only joins in the epilogue (during the run it carries exp)
            rot = (nc.sync, nc.gpsimd, nc.scalar) if bias_eng is not None \
                else (nc.sync, nc.gpsimd)
            eng = rot[out_dma_n[0] % len(rot)]
            out_dma_n[0] += 1
            eng.dma_start(out_t[(8 * g + jt) * 128 : (8 * g + jt + 1) * 128, :],
                          osb[:])

        work_q = deque()

        # Dummy zero-matmuls to keep the PE clock gate open when real filler
        # runs dry.
        hb_n = [0]

        def heartbeat(n=2, pool=None, tag="qv"):
            t = (pool or qv_ps).tile([128, 512], FP32, tag=tag,
                                     name=f"hb{hb_n[0]}")
            hb_n[0] += 1
            for i in range(n):
                nc.tensor.matmul(
                    t[:], lhsT=warm_sb[:, 0:128], rhs=warm_sb[:],
                    start=(i == 0), stop=(i == n - 1), skip_group_check=True,
                )

        def emit_item(item):
            if item[0] == "qk":
                emit_qk_group(item[1], item[2])
            elif item[0] == "v":
                emit_v_group(item[1])
            else:
                emit_proj_group(item[1], item[2])
            emitted.add(item)

        # Per-unit pop rationing: without it, all of proj(2) gets consumed
        # as filler during unit (3,0), leaving unit (3,1)'s norm/boundary
        # with nothing to cover PE stalls.
        pop_budget = [10**9]

        def pop_one(force=False):
            if work_q and pop_budget[0] > 0:
                pop_budget[0] -= 1
                emit_item(work_q.popleft())

        def drain_until(needed):
            for item in needed:
                while item not in emitted:
                    emit_item(work_q.popleft())

        # prologue: enough qkv for unit (0, 0), rest queued in dep-safe order.
        # These groups run back-to-back with no attention work between them,
        # so alternate PSUM pools (qv_ps / the still-idle bp_ps) to double-
        # buffer the group boundary: group N+1's matmuls start while DVE is
        # still reading group N's PSUM for the bias-add.
        for n, item in enumerate(
            [("qk", 0, 0), ("qk", 2, 0), ("v", 0), ("v", 1), ("v", 2),
             ("v", 3)]
        ):
            pool, tag = ((qv_ps, QTAG), (bp_ps, BTAG))[n % 2]
            if item[0] == "qk":
                emit_qk_group(item[1], item[2], pool=pool, tag=tag)
            else:
                emit_v_group(item[1], pool=pool, tag=tag)
            emitted.add(item)
        work_q.extend([("qk", 1, 0), ("qk", 3, 0)])
        for gg in range(1, NG):
            work_q.extend(
                [("qk", 2, gg), ("qk", 0, gg), ("qk", 3, gg), ("qk", 1, gg)]
                + [("v", 4 * gg + i) for i in range(4)]
            )

        # --- attention: software-pipelined units -----------------------------
        def norm_pre(g, p, av_e, av_o):
            """Stage the raw denominator rows PSUM -> SBUF bf16 (into lane 64
            of full-height tiles so the broadcast matmul stays in the default
            PE mode; other lanes hold stale-but-finite data that the zero
            weight rows annihilate)."""
            rb_e = riv_pool.tile([128, 512], BF16_DT, tag="rb", name=f"rbe{g}{p}")
            rb_o = riv_pool.tile([128, 512], BF16_DT, tag="rb", name=f"rbo{g}{p}")
            nc.vector.tensor_copy(out=rb_o[64:65, :], in_=av_o[64:65, :])
            nc.vector.tensor_copy(out=rb_e[64:65, :], in_=av_e[64:65, :])
            return rb_e, rb_o

        def norm_post(g, p, av_e, av_o, rb_e, rb_o):
            """Broadcast den across 64 partitions (128-mode matmul against the
            zero-padded ones tile), take 1/den with the custom DVE reciprocal
            (18-bit accurate, replaces the ScalarE Ln/Exp chain AND the
            PSUM->SBUF copy), then normalize.  Default: the o half goes first
            so its SBUF->SBUF relocation DMA overlaps the e half's work.
            EFIRST: e first, so av_e's PSUM buffer frees one step earlier for
            the next unit's AV chain.  Enqueues proj work for p==1."""
            tqs = slice(TQ * g, TQ * (g + 1))

            def do_o():
                bc_o = bp_ps.tile([128, 512], FP32, tag=BTAG, name=f"bco{g}{p}")
                nc.tensor.matmul(
                    bc_o[:], lhsT=zo[:, :], rhs=rb_o[:, :], start=True, stop=True,
                )
                bcs_o = bcs_pool.tile([64, 512], FP32, tag="bcs",
                                      name=f"bcso{g}{p}")
                nc.vector.reciprocal_approx_fast(out=bcs_o[:], in_=bc_o[0:64, :])
                scr = scr_pool.tile([64, 512], BF16_DT, tag="scr",
                                    name=f"scr{g}{p}")
                nc.vector.tensor_tensor(
                    out=scr[:], in0=av_o[0:64, :], in1=bcs_o[:], op=mult
                )
                nc.sync.dma_start(att_sb[p][64:128, tqs], scr[:])

            def do_e():
                bc_e = bp_ps.tile([128, 512], FP32, tag=BTAG, name=f"bce{g}{p}")
                nc.tensor.matmul(
                    bc_e[:], lhsT=zo[:, :], rhs=rb_e[:, :], start=True, stop=True,
                )
                bcs_e = bcs_pool.tile([64, 512], FP32, tag="bcs",
                                      name=f"bcse{g}{p}")
                nc.vector.reciprocal_approx_fast(out=bcs_e[:], in_=bc_e[0:64, :])
                nc.vector.tensor_tensor(
                    out=att_sb[p][0:64, tqs], in0=av_e[0:64, :], in1=bcs_e[:],
                    op=mult,
                )

            if EFIRST:
                do_e(); do_o()
            else:
                do_o(); do_e()
            if p == 1:
                # release most proj groups now, but defer a few by one unit:
                # without this, all of proj(g) is consumed as filler during
                # the NEXT unit, leaving the unit after it (e.g. (3,1)) dry.
                work_q.extend([("proj", jt, g) for jt in range(8 - DEFER)])
                defer_q.extend([("proj", jt, g) for jt in range(8 - DEFER, 8)])

        pending_norm = None
        defer_q = deque()
        units = [(0, 0), (0, 1), (1, 0), (1, 1), (2, 0), (2, 1),
                 (3, 0), (3, 1)]
        for ui, (g, p) in enumerate(units):
                # deferred proj groups from two units ago become poppable now
                work_q.extend(defer_q)
                defer_q.clear()
                # ration pops: spread remaining filler work over the
                # remaining units (the tail units otherwise run dry)
                if RATION:
                    units_left = len(units) - ui
                    pop_budget[0] = -(-len(work_q) // units_left) + 2
                nkt = 4 * (g + 1)
                h_e, h_o = 2 * p, 2 * p + 1
                q_t, k_t = qk_sb[p], qk_sb[2 + p]
                tq0 = TQ * g
                drain_until(
                    [("qk", p, g)]
                    + [("qk", 2 + p, gg) for gg in range(g + 1)]
                    + [("v", t) for t in range(nkt)]
                )
                s_tiles = {}
                p_tiles = {}
                av_e = av_o = None

                def lo_of(kt, g=g):
                    i = kt - 4 * g
                    return 128 * i if i > 0 else 0

                def scores(kt, g=g, q_t=q_t, k_t=k_t, tq0=tq0, p=p):
                    lo = lo_of(kt, g)
                    s_pair = sc_ps.tile([128, 1024], FP32, tag="sc",
                                        name=f"s{g}{p}{kt}")
                    kts = slice(128 * kt, 128 * (kt + 1))
                    rq = slice(tq0 + lo, tq0 + 512)
                    nc.tensor.matmul(
                        s_pair[:, lo:512], lhsT=k_t[0:64, kts], rhs=q_t[0:64, rq],
                        start=True, stop=True,
                    )
                    nc.tensor.matmul(
                        s_pair[:, 512 + lo : 1024], lhsT=k_t[64:128, kts],
                        rhs=q_t[64:128, rq], start=True, stop=True,
                        tile_position=(64, 0),
                    )
                    s_tiles[kt] = s_pair

                def expmask(kt, g=g, p=p):
                    lo = lo_of(kt, g)
                    s_pair = s_tiles.pop(kt)
                    p_pair = pt_pool.tile([128, 1024], BF16_DT, tag="pt",
                                          name=f"p{g}{p}{kt}")
                    s3 = s_pair[:].rearrange("p (h c) -> p h c", c=512)[:, :, lo:512]
                    p3 = p_pair[:].rearrange("p (h c) -> p h c", c=512)[:, :, lo:512]
                    nc.scalar.activation(p3, s3, Exp, scale=0.125)
                    if kt >= 4 * g:  # diagonal: mask the leading 128-wide strip
                        pm = p_pair[:].rearrange("p (h c) -> p h c", c=512)[
                            :, :, lo : lo + 128
                        ]
                        mk = maskd[:, None, 0:128].to_broadcast([128, 2, 128])
                        # DVE, not GpSimd: the gpsimd FIFO also carries the
                        # expensive SWDGE descriptor generation for the xt
                        # bulk loads, which can delay early diagonal masks
                        # (exp->AV critical path) by several us; DVE does the
                        # bf16 multiply ~3x faster on an uncontended queue
                        nc.vector.tensor_tensor(out=pm, in0=pm, in1=mk, op=mult)
                    p_tiles[kt] = p_pair

                def av_mm(kt, g=g, p=p, nkt=nkt, h_e=h_e, h_o=h_o):
                    lo = lo_of(kt, g)
                    p_pair = p_tiles.pop(kt)
                    nc.tensor.matmul(
                        av_e[:, lo:512], lhsT=v_sb[kt][:, 65 * h_e : 65 * h_e + 65],
                        rhs=p_pair[:, lo:512], start=(kt == 0),
                        stop=(kt == nkt - 1), skip_group_check=True,
                    )
                    nc.tensor.matmul(
                        av_o[:, lo:512], lhsT=v_sb[kt][:, 65 * h_o : 65 * h_o + 65],
                        rhs=p_pair[:, 512 + lo : 1024], start=(kt == 0),
                        stop=(kt == nkt - 1), skip_group_check=True,
                    )

                depth = min(DEPTH, nkt)
                # unit prologue: scores+exp for the first `depth` kts, score
                # pairs adjacent per 2-kt superstep, filler pops keeping the
                # PE fed while ScalarE works through the exps
                for k0 in range(0, depth, 2):
                    scores(k0)
                    scores(k0 + 1)
                    expmask(k0)
                    expmask(k0 + 1)
                    for _ in range(PRO_POPS):
                        pop_one()
                if pending_norm is not None:
                    rivs = norm_pre(*pending_norm)
                    for _ in range(BOUNDARY_POPS):
                        pop_one()
                    norm_post(*pending_norm, *rivs)
                av_e = av_ps.tile([65, 512], FP32, tag="av", name=f"ave{g}{p}")
                av_o = av_ps.tile([65, 512], FP32, tag="av", name=f"avo{g}{p}")
                for k0 in range(depth, nkt, 2):
                    scores(k0)
                    scores(k0 + 1)
                    expmask(k0)
                    expmask(k0 + 1)
                    av_mm(k0 - depth)
                    for _ in range(POPS_EVERY):
                        pop_one()
                    av_mm(k0 - depth + 1)
                for kt in range(nkt - depth, nkt, 2):
                    av_mm(kt)
                    pop_one()
                    av_mm(kt + 1)
                pending_norm = (g, p, av_e, av_o)

        # epilogue: last unit's normalize + remaining proj groups, rotating
        # the proj PSUM through the now-idle score/qkv banks for overlap
        rivs = norm_pre(*pending_norm)
        norm_post(*pending_norm, *rivs)
        work_q.extend(defer_q)
        defer_q.clear()
        ep_n = 0
        ep_rot = [(sc_ps, "sc"), (bp_ps, BTAG), (qv_ps, QTAG)]
        ep_bias = [nc.vector, nc.scalar]
        while work_q:
            item = work_q.popleft()
            if item[0] == "proj":
                pool, tag = ep_rot[ep_n % 3]
                emit_proj_group(item[1], item[2], pool=pool, tag=tag,
                                bias_eng=ep_bias[ep_n % 2])
                ep_n += 1
                emitted.add(item)
            else:
                emit_item(item)

    nc.compile()
    return nc


_NC_CACHE = None


def _get_nc():
    global _NC_CACHE
    if _NC_CACHE is None:
        _NC_CACHE = build_bass()
    return _NC_CACHE


def make_in_maps(x, w_attn, b_attn, w_proj, b_proj):
    """Host-side sharding: slice/transpose/cast the full inputs per core."""
    x = np.asarray(x, dtype=np.float32)
    w_attn = np.asarray(w_attn, dtype=np.float32)
    b_attn = np.asarray(b_attn, dtype=np.float32)
    w_proj = np.asarray(w_proj, dtype=np.float32)
    b_proj = np.asarray(b_proj, dtype=np.float32)
    in_maps = []
    for core in range(N_CORES):
        b = core // 4
        heads = [4 * (core % 4) + i for i in range(HEADS_PER_CORE)]
        ch = np.concatenate([np.arange(h * DH, (h + 1) * DH) for h in heads])
        idx_qk = np.concatenate([ch, C + ch])
        idx_v = 2 * C + ch
        # xt blocked: tile (chunk i, tq block g) contiguous at rows (4i+g)*128
        xt_full = x[b].T  # [C, T]
        xt_blk = np.ascontiguousarray(
            xt_full.reshape(NC_, 128, 4, 512).transpose(0, 2, 1, 3)
        ).reshape(NC_ * 4 * 128, 512)
        # wqkt blocked halves: tile (chunk i, half h) at rows (2i+h)*128 with
        # cols [q heads (2h,2h+1) | k heads (2h,2h+1)]
        wqkt_full = w_attn[idx_qk].T  # [C, 512] cols = [q01 q23 k01 k23]
        wq = wqkt_full.reshape(NC_, 128, 4, 128)
        wqk_blk = np.stack(
            [np.concatenate([wq[:, :, h], wq[:, :, 2 + h]], axis=2)
             for h in range(2)], axis=1,
        ).reshape(NC_ * 2 * 128, 256)
        in_maps.append(
            {
                "xt": np.ascontiguousarray(xt_blk).astype(BF16),
                "wqkt": np.ascontiguousarray(wqk_blk).astype(BF16),
                "wvt": np.ascontiguousarray(w_attn[idx_v].T).astype(BF16),
                "wpt": np.ascontiguousarray(w_proj[:, ch].T).astype(BF16),
                "bqk": b_attn[idx_qk].astype(np.float32).reshape(512, 1),
                "bv": np.tile(b_attn[idx_v].astype(np.float32)[None, :], (128, 1)),
                "bp": (b_proj / 4.0).astype(np.float32).reshape(C, 1),
            }
        )
    return in_maps


def assemble_output(results):
    out = np.zeros((B, T, C), dtype=np.float32)
    for core in range(N_CORES):
        # blocked out_t: tile (g, jt) at rows (8g+jt)*128 holds
        # out[b, 512g:512(g+1), 128jt:128(jt+1)].T
        arr = np.asarray(results[core]["out_t"]).astype(np.float32)
        arr = arr.reshape(4, 8, 128, 512).transpose(0, 3, 1, 2).reshape(T, C)
        out[core // 4] += arr
    return out


def run(inputs, trace=False, trace_cores=None, tmpdir=None):
    """Run on hardware; returns (output, BassKernelResults)."""
    _ensure_axon_hooks_stub()
    from concourse.bass_utils import run_bass_kernel_spmd

    nc = _get_nc()
    in_maps = make_in_maps(**inputs)
    kw = {}
    if trace:
        kw.update(trace=True, trace_cores=trace_cores, tmpdir=tmpdir)
    res = run_bass_kernel_spmd(nc, in_maps, core_ids=list(range(N_CORES)), **kw)
    return assemble_output(res.results), res


def kernel(x, w_attn, b_attn, w_proj, b_proj):
    out, _ = run(
        dict(x=x, w_attn=w_attn, b_attn=b_attn, w_proj=w_proj, b_proj=b_proj)
    )
    return out

